# revision 3
# baseline (speedup 1.0000x reference)
import sys

sys.path.insert(0, "/opt/trn_rl_repo")

import numpy as np
import concourse.bass as bass  # noqa: F401  (registers types)
from concourse import bacc
import concourse.mybir as mybir
from concourse.tile import TileContext
from concourse.bass_utils import run_bass_kernel_spmd

S = 4096          # sequence length
D = 1024          # model/key/value dim
NCORES = 8
R = S // NCORES   # 512 rows per core
KC = D // 128     # 8 contraction chunks
J = S // 128      # 32 key tiles
VA = D + 2        # V augmented with ones column (denominator) + zero pad (fp32r even-size rule)
CH = [(0, 342), (342, 342), (684, 342)]  # PV output column chunks (<=512 moving, >=256, even)

F32 = mybir.dt.float32
F32R = mybir.dt.float32r

_cache = {}


def _build_phase1():
    """Per core: qT = (Wq/32)^T-proj, kT, v from its 512-row slice of x.

    Inputs (pre-chunked so SBUF partition dim is 128):
      xsT [128, KC*R]: [p, k*R+i] = x[i, 128k+p]   (x slice transposed)
      wq/wk/wv [128, KC*D]: [p, k*D+d] = W[128k+p, d]
    Outputs: qT/kT [D, R] (feature-major), v [R, D]. Biases added on host.
    """
    nc = bacc.Bacc(None, target_bir_lowering=False)
    xsT = nc.dram_tensor("xsT", [128, KC * R], F32R, kind="ExternalInput")
    wq = nc.dram_tensor("wq", [128, KC * D], F32R, kind="ExternalInput")
    wk = nc.dram_tensor("wk", [128, KC * D], F32R, kind="ExternalInput")
    wv = nc.dram_tensor("wv", [128, KC * D], F32R, kind="ExternalInput")
    qT = nc.dram_tensor("qT", [D, R], F32, kind="ExternalOutput")
    kT = nc.dram_tensor("kT", [D, R], F32, kind="ExternalOutput")
    v = nc.dram_tensor("v", [R, D], F32, kind="ExternalOutput")
    with TileContext(nc) as tc:
        with tc.tile_pool(name="inp", bufs=1) as inp, \
             tc.tile_pool(name="ob", bufs=4) as ob, \
             tc.tile_pool(name="ps", bufs=4, space="PSUM") as ps:
            xt = inp.tile([128, KC * R], F32R)
            nc.sync.dma_start(xt[:], xsT[:])
            wts = []
            for nm, src in (("wqt", wq), ("wkt", wk), ("wvt", wv)):
                w = inp.tile([128, KC * D], F32R, name=nm)
                nc.sync.dma_start(w[:], src[:])
                wts.append(w)
            # qT/kT: out[d, i] = sum_k W[k, d] * x[i, k] -> lhsT=W chunk, rhs=xT chunk
            for wt, outT in ((wts[0], qT), (wts[1], kT)):
                for d in range(KC):
                    p = ps.tile([128, R], F32, name=f"p_{outT.name}_{d}", tag="ps")
                    for k in range(KC):
                        nc.tensor.matmul(
                            p[:],
                            wt[:, k * D + d * 128 : k * D + d * 128 + 128],
                            xt[:, k * R : (k + 1) * R],
                            start=(k == 0), stop=(k == KC - 1),
                        )
                    o = ob.tile([128, R], F32, name=f"o_{outT.name}_{d}", tag="ob")
                    nc.vector.tensor_copy(o[:], p[:])
                    nc.sync.dma_start(outT[d * 128 : (d + 1) * 128, :], o[:])
            # v: out[i, n] = sum_k x[i, k] * Wv[k, n]
            for i in range(R // 128):
                for n2 in range(2):
                    p = ps.tile([128, 512], F32, name=f"pv_{i}_{n2}", tag="ps")
                    for k in range(KC):
                        nc.tensor.matmul(
                            p[:],
                            xt[:, k * R + i * 128 : k * R + i * 128 + 128],
                            wts[2][:, k * D + n2 * 512 : k * D + (n2 + 1) * 512],
                            start=(k == 0), stop=(k == KC - 1),
                        )
                    o = ob.tile([128, 512], F32, name=f"ov_{i}_{n2}", tag="ob")
                    nc.vector.tensor_copy(o[:], p[:])
                    nc.sync.dma_start(v[i * 128 : (i + 1) * 128, n2 * 512 : (n2 + 1) * 512], o[:])
    nc.finalize()
    return nc


def _build_phase2():
    """Per core: anti-causal attention for its 512 query rows vs all 4096 keys.

    Scores computed transposed (S^T[j,i], keys on partitions), masked+exp'd via
    iota<=thr data mask, then P^T @ V_aug accumulated in PSUM over j in two
    16-tile halves; ones column of V_aug yields the softmax denominator.
      qt [128, KC*R]: [p, k*R+i] = qT[128k+p, i]   (q pre-scaled by 1/sqrt(D))
      kt [128, J*D]:  [p, j*D + k*128 + c] = kT[128k+p, 128j+c]
      vi [128, J*VA]: [p, j*VA + c] = v_aug[128j+p, c]
      io [128, R]: iota row (0..R-1), th [128, J]: thr[p,j] = 128j+p-512*core
    Output rd [R, D] = normalized attention read.
    """
    nc = bacc.Bacc(None, target_bir_lowering=False)
    qt_in = nc.dram_tensor("qt", [128, KC * R], F32R, kind="ExternalInput")
    kt_in = nc.dram_tensor("kt", [128, J * D], F32R, kind="ExternalInput")
    v_in = nc.dram_tensor("vi", [128, J * VA], F32R, kind="ExternalInput")
    iota = nc.dram_tensor("io", [128, R], F32, kind="ExternalInput")
    thr = nc.dram_tensor("th", [128, J], F32, kind="ExternalInput")
    rd = nc.dram_tensor("rd", [R, D], F32, kind="ExternalOutput")
    NI = R // 128  # 4 query chunks
    with TileContext(nc) as tc:
        with tc.tile_pool(name="pp", bufs=J) as ppool, \
             tc.tile_pool(name="ac", bufs=3 * NI) as ac, \
             tc.tile_pool(name="no", bufs=4) as no:
            pts = []
            # ---- scores + exp + mask pass ----
            with tc.tile_pool(name="cst", bufs=1) as cst, \
                 tc.tile_pool(name="kp", bufs=3) as kp, \
                 tc.tile_pool(name="sp", bufs=2, space="PSUM") as sp, \
                 tc.tile_pool(name="ep", bufs=3) as ep:
                qt = cst.tile([128, KC * R], F32R)
                nc.sync.dma_start(qt[:], qt_in[:])
                io = cst.tile([128, R], F32)
                nc.sync.dma_start(io[:], iota[:])
                th = cst.tile([128, J], F32)
                nc.sync.dma_start(th[:], thr[:])
                for j in range(J):
                    kt = kp.tile([128, D], F32R, name=f"kt{j}", tag="kt")
                    nc.sync.dma_start(kt[:], kt_in[:, j * D : (j + 1) * D])
                    ps_ = sp.tile([128, R], F32, name=f"s{j}", tag="s")
                    for k in range(KC):
                        nc.tensor.matmul(
                            ps_[:],
                            kt[:, k * 128 : (k + 1) * 128],
                            qt[:, k * R : (k + 1) * R],
                            start=(k == 0), stop=(k == KC - 1),
                        )
                    ex = ep.tile([128, R], F32, name=f"e{j}", tag="e")
                    nc.scalar.activation(ex[:], ps_[:], mybir.ActivationFunctionType.Exp)
                    pt = ppool.tile([128, R], F32R, name=f"pt{j}", tag="pt")
                    nc.vector.scalar_tensor_tensor(
                        pt[:], io[:], th[:, j : j + 1], ex[:],
                        op0=mybir.AluOpType.is_le, op1=mybir.AluOpType.mult,
                    )
                    pts.append(pt)
            # ---- P^T @ V_aug in two j-halves (16 V tiles resident each) ----
            accs = {}
            for h in range(2):
                with tc.tile_pool(name=f"vp{h}", bufs=J // 2) as vp, \
                     tc.tile_pool(name=f"p2{h}", bufs=3, space="PSUM") as p2:
                    vts = []
                    for jj in range(J // 2):
                        j = h * (J // 2) + jj
                        vt = vp.tile([128, VA], F32R, name=f"vt{j}", tag="vt")
                        nc.sync.dma_start(vt[:], v_in[:, j * VA : (j + 1) * VA])
                        vts.append(vt)
                    for i in range(NI):
                        for cidx, (c0, w) in enumerate(CH):
                            pz = p2.tile([128, w], F32, name=f"pv{h}_{i}_{cidx}", tag="pv")
                            for jj in range(J // 2):
                                j = h * (J // 2) + jj
                                nc.tensor.matmul(
                                    pz[:],
                                    pts[j][:, i * 128 : (i + 1) * 128],
                                    vts[jj][:, c0 : c0 + w],
                                    start=(jj == 0), stop=(jj == J // 2 - 1),
                                )
                            if h == 0:
                                a_ = ac.tile([128, w], F32, name=f"ac{i}_{cidx}", tag="ac")
                                accs[(i, cidx)] = a_
                                nc.vector.tensor_copy(a_[:], pz[:])
                            else:
                                a_ = accs[(i, cidx)]
                                nc.vector.tensor_add(a_[:], a_[:], pz[:])
            # ---- normalize by ones-column denominator and write out ----
            for i in range(NI):
                rec = no.tile([128, 1], F32, name=f"rc{i}", tag="rc")
                nc.vector.reciprocal(rec[:], accs[(i, 2)][:, 340:341])
                for cidx, (c0, w) in enumerate(CH):
                    wo = w if cidx != 2 else 340  # drop the ones column
                    o = no.tile([128, 342], F32, name=f"o{i}_{cidx}", tag="o")
                    nc.vector.tensor_scalar_mul(o[:, :wo], accs[(i, cidx)][:, :wo], rec[:])
                    nc.sync.dma_start(rd[i * 128 : (i + 1) * 128, c0 : c0 + wo], o[:, :wo])
    nc.finalize()
    return nc


def _chunk_rows(a, nchunks):
    # [Nchunks*128, C] -> [128, nchunks*C] with [p, k*C+c] = a[128k+p, c]
    n, c = a.shape
    assert n == nchunks * 128
    return np.ascontiguousarray(
        a.reshape(nchunks, 128, c).transpose(1, 0, 2).reshape(128, nchunks * c)
    )


def kernel(x, Wk, bk, Wq, bq, Wv, bv):
    x = np.asarray(x, dtype=np.float32)
    Wk = np.asarray(Wk, dtype=np.float32)
    Wq = np.asarray(Wq, dtype=np.float32)
    Wv = np.asarray(Wv, dtype=np.float32)
    bk = np.asarray(bk, dtype=np.float32)
    bq = np.asarray(bq, dtype=np.float32)
    bv = np.asarray(bv, dtype=np.float32)

    sc = np.float32(1.0 / np.sqrt(D))
    if "p1" not in _cache:
        _cache["p1"] = _build_phase1()
    if "p2" not in _cache:
        _cache["p2"] = _build_phase2()

    wq_in = _chunk_rows(Wq * sc, KC)
    wk_in = _chunk_rows(Wk, KC)
    wv_in = _chunk_rows(Wv, KC)
    in_maps1 = []
    for c in range(NCORES):
        xs = x[c * R : (c + 1) * R]
        xsT_in = _chunk_rows(np.ascontiguousarray(xs.T), KC)
        in_maps1.append({"xsT": xsT_in, "wq": wq_in, "wk": wk_in, "wv": wv_in})
    res1 = run_bass_kernel_spmd(_cache["p1"], in_maps1, list(range(NCORES))).results

    bq_s = (bq * sc)[:, None]
    qTs = [res1[c]["qT"] + bq_s for c in range(NCORES)]
    kT_g = np.concatenate([res1[c]["kT"] for c in range(NCORES)], axis=1) + bk[:, None]
    v_g = np.concatenate([res1[c]["v"] for c in range(NCORES)], axis=0) + bv[None, :]
    v_aug = np.concatenate(
        [v_g, np.ones((S, 1), np.float32), np.zeros((S, 1), np.float32)], axis=1
    )

    # kt layout: [p, j, k, c] = kT_g[128k+p, 128j+c]
    kt_in = np.ascontiguousarray(
        kT_g.reshape(KC, 128, J, 128).transpose(1, 2, 0, 3).reshape(128, J * D)
    )
    v_in = _chunk_rows(v_aug, J)
    io_in = np.ascontiguousarray(
        np.broadcast_to(np.arange(R, dtype=np.float32), (128, R))
    )
    p_idx = np.arange(128, dtype=np.float32)[:, None]
    j_idx = np.arange(J, dtype=np.float32)[None, :]
    in_maps2 = []
    for c in range(NCORES):
        thr_c = np.ascontiguousarray(128.0 * j_idx + p_idx - 512.0 * c)
        in_maps2.append({
            "qt": _chunk_rows(qTs[c], KC),
            "kt": kt_in,
            "vi": v_in,
            "io": io_in,
            "th": thr_c.astype(np.float32),
        })
    res2 = run_bass_kernel_spmd(_cache["p2"], in_maps2, list(range(NCORES))).results

    read = np.concatenate([res2[c]["rd"] for c in range(NCORES)], axis=0)
    return np.concatenate([x, read], axis=1)


# revision 4
# speedup vs baseline: 1.1186x; 1.1186x over previous
import sys

sys.path.insert(0, "/opt/trn_rl_repo")

import numpy as np
import concourse.bass as bass  # noqa: F401  (registers types)
from concourse import bacc
import concourse.mybir as mybir
from concourse.tile import TileContext
from concourse.bass_utils import run_bass_kernel_spmd

S = 4096          # sequence length
D = 1024          # model/key/value dim
NCORES = 8
R = S // NCORES   # 512 rows per core
KC = D // 128     # 8 contraction chunks
J = S // 128      # 32 key tiles
VA = D + 2        # V augmented with ones column (denominator) + zero pad (fp32r even-size rule)
CH = [(0, 342), (342, 342), (684, 342)]  # PV output column chunks (<=512 moving, >=256, even)
JQ = 8            # key tiles per PV quarter

F32 = mybir.dt.float32
F32R = mybir.dt.float32r

_cache = {}


def _build_phase1():
    """Per core: q = xs@(Wq/sqrt(D)), k = xs@Wk, v = xs@Wv for its 512-row x slice.

    One weight-load of each x chunk feeds 6 matmuls (3 projections x 2 column
    halves). Biases added on host.
      xsT [128, KC*R]: [p, k*R+i] = x[i, 128k+p]
      wq/wk/wv [128, KC*D]: [p, k*D+d] = W[128k+p, d]
    Outputs: q/k/v [R, D] natural layout.
    """
    nc = bacc.Bacc(None, target_bir_lowering=False)
    xsT = nc.dram_tensor("xsT", [128, KC * R], F32R, kind="ExternalInput")
    wins = [nc.dram_tensor(n, [128, KC * D], F32R, kind="ExternalInput")
            for n in ("wq", "wk", "wv")]
    outs = [nc.dram_tensor(n, [R, D], F32, kind="ExternalOutput") for n in ("q", "k", "v")]
    with TileContext(nc) as tc:
        with tc.tile_pool(name="inp", bufs=1) as inp, \
             tc.tile_pool(name="ob", bufs=6) as ob, \
             tc.tile_pool(name="ps", bufs=6, space="PSUM") as ps:
            xt = inp.tile([128, KC * R], F32R)
            for k in range(KC):
                nc.sync.dma_start(xt[:, k * R : (k + 1) * R], xsT[:, k * R : (k + 1) * R])
            wts = []
            for w_i, src in enumerate(wins):
                w = inp.tile([128, KC * D], F32R, name=f"w{w_i}")
                for k in range(KC):
                    nc.sync.dma_start(w[:, k * D : (k + 1) * D], src[:, k * D : (k + 1) * D])
                wts.append(w)
            for i in range(R // 128):
                pz = {}
                for w_i in range(3):
                    for n2 in range(2):
                        pz[(w_i, n2)] = ps.tile([128, 512], F32, name=f"p{i}_{w_i}_{n2}", tag="ps")
                for k in range(KC):
                    lhsT = xt[:, k * R + i * 128 : k * R + i * 128 + 128]
                    for w_i in range(3):
                        for n2 in range(2):
                            nc.tensor.matmul(
                                pz[(w_i, n2)][:],
                                lhsT,
                                wts[w_i][:, k * D + n2 * 512 : k * D + (n2 + 1) * 512],
                                start=(k == 0), stop=(k == KC - 1),
                            )
                for w_i in range(3):
                    for n2 in range(2):
                        o = ob.tile([128, 512], F32, name=f"o{i}_{w_i}_{n2}", tag="ob")
                        nc.vector.tensor_copy(o[:], pz[(w_i, n2)][:])
                        nc.sync.dma_start(
                            outs[w_i][i * 128 : (i + 1) * 128, n2 * 512 : (n2 + 1) * 512], o[:]
                        )
    nc.finalize()
    return nc


def _build_phase2():
    """Per core: anti-causal attention for its 512 query rows vs all 4096 keys.

    Scores computed transposed (S^T[j,i], keys on partitions), masked+exp'd via
    an iota<=thr data mask. P^T @ V_aug accumulates over j in PSUM per quarter
    (8 j-tiles), with one P^T weight-load per (i, j) feeding 3 column chunks.
    The ones column of V_aug yields the softmax denominator.
      qt [128, KC*R]: [p, k*R+i] = qT[128k+p, i]   (q pre-scaled by 1/sqrt(D))
      kt [128, J*D]:  [p, j*D + k*128 + c] = kT[128k+p, 128j+c]
      vi [128, J*VA]: [p, j*VA + c] = v_aug[128j+p, c]
      io [128, R]: iota row (0..R-1), th [128, J]: thr[p,j] = 128j+p-512*core
    Output rd [R, D] = normalized attention read.
    """
    nc = bacc.Bacc(None, target_bir_lowering=False)
    qt_in = nc.dram_tensor("qt", [128, KC * R], F32R, kind="ExternalInput")
    kt_in = nc.dram_tensor("kt", [128, J * D], F32R, kind="ExternalInput")
    v_in = nc.dram_tensor("vi", [128, J * VA], F32R, kind="ExternalInput")
    iota = nc.dram_tensor("io", [128, R], F32, kind="ExternalInput")
    thr = nc.dram_tensor("th", [128, J], F32, kind="ExternalInput")
    rd = nc.dram_tensor("rd", [R, D], F32, kind="ExternalOutput")
    NI = R // 128  # 4 query chunks
    NQ = J // JQ   # 4 quarters
    with TileContext(nc) as tc:
        with tc.tile_pool(name="cst", bufs=1) as cst, \
             tc.tile_pool(name="kp", bufs=3) as kp, \
             tc.tile_pool(name="sp", bufs=2, space="PSUM") as sp, \
             tc.tile_pool(name="ep", bufs=3) as ep, \
             tc.tile_pool(name="pp", bufs=J) as ppool, \
             tc.tile_pool(name="vp", bufs=JQ + 4) as vp, \
             tc.tile_pool(name="p2", bufs=6, space="PSUM") as p2, \
             tc.tile_pool(name="ac", bufs=3 * NI) as ac, \
             tc.tile_pool(name="no", bufs=4) as no:
            qt = cst.tile([128, KC * R], F32R)
            for k in range(KC):
                nc.sync.dma_start(qt[:, k * R : (k + 1) * R], qt_in[:, k * R : (k + 1) * R])
            io = cst.tile([128, R], F32)
            nc.sync.dma_start(io[:], iota[:])
            th = cst.tile([128, J], F32)
            nc.sync.dma_start(th[:], thr[:])
            # ---- scores + exp + mask pass ----
            pts = []
            for j in range(J):
                kt = kp.tile([128, D], F32R, name=f"kt{j}", tag="kt")
                nc.sync.dma_start(kt[:], kt_in[:, j * D : (j + 1) * D])
                ps_ = sp.tile([128, R], F32, name=f"s{j}", tag="s")
                for k in range(KC):
                    nc.tensor.matmul(
                        ps_[:],
                        kt[:, k * 128 : (k + 1) * 128],
                        qt[:, k * R : (k + 1) * R],
                        start=(k == 0), stop=(k == KC - 1),
                    )
                ex = ep.tile([128, R], F32, name=f"e{j}", tag="e")
                nc.scalar.activation(ex[:], ps_[:], mybir.ActivationFunctionType.Exp)
                pt = ppool.tile([128, R], F32R, name=f"pt{j}", tag="pt")
                nc.vector.scalar_tensor_tensor(
                    pt[:], io[:], th[:, j : j + 1], ex[:],
                    op0=mybir.AluOpType.is_le, op1=mybir.AluOpType.mult,
                )
                pts.append(pt)
            # ---- P^T @ V_aug, PSUM-accumulated per quarter ----
            accs = {}
            for q in range(NQ):
                vts = []
                for jj in range(JQ):
                    j = q * JQ + jj
                    vt = vp.tile([128, VA], F32R, name=f"vt{j}", tag="vt")
                    nc.sync.dma_start(vt[:], v_in[:, j * VA : (j + 1) * VA])
                    vts.append(vt)
                for i in range(NI):
                    pz = [p2.tile([128, w], F32, name=f"pv{q}_{i}_{c}", tag="pv")
                          for c, (_, w) in enumerate(CH)]
                    for jj in range(JQ):
                        j = q * JQ + jj
                        for cidx, (c0, w) in enumerate(CH):
                            nc.tensor.matmul(
                                pz[cidx][:],
                                pts[j][:, i * 128 : (i + 1) * 128],
                                vts[jj][:, c0 : c0 + w],
                                start=(jj == 0), stop=(jj == JQ - 1),
                            )
                    for cidx in range(3):
                        if q == 0:
                            a_ = ac.tile([128, CH[cidx][1]], F32, name=f"ac{i}_{cidx}", tag="ac")
                            accs[(i, cidx)] = a_
                            nc.vector.tensor_copy(a_[:], pz[cidx][:])
                        else:
                            a_ = accs[(i, cidx)]
                            nc.vector.tensor_add(a_[:], a_[:], pz[cidx][:])
            # ---- normalize by ones-column denominator and write out ----
            for i in range(NI):
                rec = no.tile([128, 1], F32, name=f"rc{i}", tag="rc")
                nc.vector.reciprocal(rec[:], accs[(i, 2)][:, 340:341])
                for cidx, (c0, w) in enumerate(CH):
                    wo = w if cidx != 2 else 340  # drop the ones/pad columns
                    o = no.tile([128, 342], F32, name=f"o{i}_{cidx}", tag="o")
                    nc.vector.tensor_scalar_mul(o[:, :wo], accs[(i, cidx)][:, :wo], rec[:])
                    nc.sync.dma_start(rd[i * 128 : (i + 1) * 128, c0 : c0 + wo], o[:, :wo])
    nc.finalize()
    return nc


def _chunk_rows(a, nchunks):
    # [nchunks*128, C] -> [128, nchunks*C] with [p, k*C+c] = a[128k+p, c]
    n, c = a.shape
    assert n == nchunks * 128
    return np.ascontiguousarray(
        a.reshape(nchunks, 128, c).transpose(1, 0, 2).reshape(128, nchunks * c)
    )


def kernel(x, Wk, bk, Wq, bq, Wv, bv):
    x = np.asarray(x, dtype=np.float32)
    Wk = np.asarray(Wk, dtype=np.float32)
    Wq = np.asarray(Wq, dtype=np.float32)
    Wv = np.asarray(Wv, dtype=np.float32)
    bk = np.asarray(bk, dtype=np.float32)
    bq = np.asarray(bq, dtype=np.float32)
    bv = np.asarray(bv, dtype=np.float32)

    sc = np.float32(1.0 / np.sqrt(D))
    if "p1" not in _cache:
        _cache["p1"] = _build_phase1()
    if "p2" not in _cache:
        _cache["p2"] = _build_phase2()

    wq_in = _chunk_rows(Wq * sc, KC)
    wk_in = _chunk_rows(Wk, KC)
    wv_in = _chunk_rows(Wv, KC)
    in_maps1 = []
    for c in range(NCORES):
        xs = x[c * R : (c + 1) * R]
        xsT_in = _chunk_rows(np.ascontiguousarray(xs.T), KC)
        in_maps1.append({"xsT": xsT_in, "wq": wq_in, "wk": wk_in, "wv": wv_in})
    res1 = run_bass_kernel_spmd(_cache["p1"], in_maps1, list(range(NCORES))).results

    bq_s = (bq * sc)[None, :]
    qs = [res1[c]["q"] + bq_s for c in range(NCORES)]
    k_g = np.concatenate([res1[c]["k"] for c in range(NCORES)], axis=0) + bk[None, :]
    v_g = np.concatenate([res1[c]["v"] for c in range(NCORES)], axis=0) + bv[None, :]
    kT_g = np.ascontiguousarray(k_g.T)  # [D, S]
    v_aug = np.concatenate(
        [v_g, np.ones((S, 1), np.float32), np.zeros((S, 1), np.float32)], axis=1
    )

    # kt layout: [p, j, k, c] = kT_g[128k+p, 128j+c]
    kt_in = np.ascontiguousarray(
        kT_g.reshape(KC, 128, J, 128).transpose(1, 2, 0, 3).reshape(128, J * D)
    )
    v_in = _chunk_rows(v_aug, J)
    io_in = np.ascontiguousarray(
        np.broadcast_to(np.arange(R, dtype=np.float32), (128, R))
    )
    p_idx = np.arange(128, dtype=np.float32)[:, None]
    j_idx = np.arange(J, dtype=np.float32)[None, :]
    in_maps2 = []
    for c in range(NCORES):
        thr_c = np.ascontiguousarray(128.0 * j_idx + p_idx - 512.0 * c).astype(np.float32)
        in_maps2.append({
            "qt": _chunk_rows(np.ascontiguousarray(qs[c].T), KC),
            "kt": kt_in,
            "vi": v_in,
            "io": io_in,
            "th": thr_c,
        })
    res2 = run_bass_kernel_spmd(_cache["p2"], in_maps2, list(range(NCORES))).results

    read = np.concatenate([res2[c]["rd"] for c in range(NCORES)], axis=0)
    return np.concatenate([x, read], axis=1)


# revision 8
# speedup vs baseline: 1.1578x; 1.0350x over previous
import sys

sys.path.insert(0, "/opt/trn_rl_repo")

import numpy as np
import concourse.bass as bass  # noqa: F401  (registers types)
from concourse import bacc
import concourse.mybir as mybir
from concourse.tile import TileContext
from concourse.bass_utils import run_bass_kernel_spmd

S = 4096          # sequence length
D = 1024          # model/key/value dim
NCORES = 8
R = S // NCORES   # 512 rows per core
KC = D // 128     # 8 contraction chunks
J = S // 128      # 32 key tiles
VA = D + 2        # V augmented with ones column (denominator) + zero pad (fp32r even-size rule)
CH = [(0, 342), (342, 342), (684, 342)]  # PV output column chunks (<=512 moving, >=256, even)
JQ = 8            # key tiles per PV quarter

F32 = mybir.dt.float32
F32R = mybir.dt.float32r

_cache = {}


def _build_phase1():
    """Per core: q = xs@(Wq/sqrt(D)), k = xs@Wk, v = xs@Wv for its 512-row x slice.

    One weight-load of each x chunk feeds 6 matmuls (3 projections x 2 column
    halves). Biases added on host.
      xsT [128, KC*R]: [p, k*R+i] = x[i, 128k+p]
      wq/wk/wv [128, KC*D]: [p, k*D+d] = W[128k+p, d]
    Outputs: q/k/v [R, D] natural layout.
    """
    nc = bacc.Bacc(None, target_bir_lowering=False)
    xsT = nc.dram_tensor("xsT", [128, KC * R], F32R, kind="ExternalInput")
    wins = [nc.dram_tensor(n, [128, KC * D], F32R, kind="ExternalInput")
            for n in ("wq", "wk", "wv")]
    outs = [nc.dram_tensor(n, [R, D], F32, kind="ExternalOutput") for n in ("q", "k", "v")]
    with TileContext(nc) as tc:
        with tc.tile_pool(name="inp", bufs=1) as inp, \
             tc.tile_pool(name="ob", bufs=6) as ob, \
             tc.tile_pool(name="ps", bufs=6, space="PSUM") as ps:
            xt = inp.tile([128, KC * R], F32R)
            for k in range(KC):
                nc.sync.dma_start(xt[:, k * R : (k + 1) * R], xsT[:, k * R : (k + 1) * R])
            wts = []
            for w_i, src in enumerate(wins):
                w = inp.tile([128, KC * D], F32R, name=f"w{w_i}")
                for k in range(KC):
                    nc.sync.dma_start(w[:, k * D : (k + 1) * D], src[:, k * D : (k + 1) * D])
                wts.append(w)
            for i in range(R // 128):
                pz = {}
                for w_i in range(3):
                    for n2 in range(2):
                        pz[(w_i, n2)] = ps.tile([128, 512], F32, name=f"p{i}_{w_i}_{n2}", tag="ps")
                for k in range(KC):
                    lhsT = xt[:, k * R + i * 128 : k * R + i * 128 + 128]
                    for w_i in range(3):
                        for n2 in range(2):
                            nc.tensor.matmul(
                                pz[(w_i, n2)][:],
                                lhsT,
                                wts[w_i][:, k * D + n2 * 512 : k * D + (n2 + 1) * 512],
                                start=(k == 0), stop=(k == KC - 1),
                            )
                for w_i in range(3):
                    for n2 in range(2):
                        o = ob.tile([128, 512], F32, name=f"o{i}_{w_i}_{n2}", tag="ob")
                        nc.vector.tensor_copy(o[:], pz[(w_i, n2)][:])
                        nc.sync.dma_start(
                            outs[w_i][i * 128 : (i + 1) * 128, n2 * 512 : (n2 + 1) * 512], o[:]
                        )
    nc.finalize()
    return nc


def _build_phase2():
    """Per core: anti-causal attention for its 512 query rows vs all 4096 keys.

    Scores computed transposed (S^T[j,i], keys on partitions), masked+exp'd via
    an iota<=thr data mask. P^T @ V_aug accumulates over j in PSUM per quarter
    (8 j-tiles), with one P^T weight-load per (i, j) feeding 3 column chunks.
    The ones column of V_aug yields the softmax denominator.
      qt [128, KC*R]: [p, k*R+i] = qT[128k+p, i]   (q pre-scaled by 1/sqrt(D))
      kt [128, J*D]:  [p, j*D + k*128 + c] = kT[128k+p, 128j+c]
      vi [128, J*VA]: [p, j*VA + c] = v_aug[128j+p, c]
      io [128, R]: iota row (0..R-1), th [128, J]: thr[p,j] = 128j+p-512*core
    Output rd [R, D] = normalized attention read.
    """
    nc = bacc.Bacc(None, target_bir_lowering=False)
    qt_in = nc.dram_tensor("qt", [128, KC * R], F32R, kind="ExternalInput")
    kt_in = nc.dram_tensor("kt", [128, J * D], F32R, kind="ExternalInput")
    v_in = nc.dram_tensor("vi", [128, J * VA], F32R, kind="ExternalInput")
    iota = nc.dram_tensor("io", [128, R], F32, kind="ExternalInput")
    thr = nc.dram_tensor("th", [128, J], F32, kind="ExternalInput")
    rdT = nc.dram_tensor("rdT", [D + 128, R], F32, kind="ExternalOutput")
    NN = D // 128  # 8 output feature chunks
    NQ = J // JQ   # 4 quarters
    with TileContext(nc) as tc:
        with tc.tile_pool(name="cst", bufs=1) as cst, \
             tc.tile_pool(name="kp", bufs=3) as kp, \
             tc.tile_pool(name="sp", bufs=2, space="PSUM") as sp, \
             tc.tile_pool(name="ep", bufs=3) as ep, \
             tc.tile_pool(name="pp", bufs=J) as ppool, \
             tc.tile_pool(name="vp", bufs=JQ + 4) as vp, \
             tc.tile_pool(name="p2", bufs=6, space="PSUM") as p2, \
             tc.tile_pool(name="ac", bufs=NN + 1) as ac, \
             tc.tile_pool(name="no", bufs=5) as no:
            qt = cst.tile([128, KC * R], F32R)
            for k in range(KC):
                nc.sync.dma_start(qt[:, k * R : (k + 1) * R], qt_in[:, k * R : (k + 1) * R])
            io = cst.tile([128, R], F32)
            nc.sync.dma_start(io[:], iota[:])
            th = cst.tile([128, J], F32)
            nc.sync.dma_start(th[:], thr[:])
            # ---- scores + exp + mask pass ----
            pts = []
            for j in range(J):
                kt = kp.tile([128, D], F32R, name=f"kt{j}", tag="kt")
                nc.sync.dma_start(kt[:], kt_in[:, j * D : (j + 1) * D])
                ps_ = sp.tile([128, R], F32, name=f"s{j}", tag="s")
                for k in range(KC):
                    nc.tensor.matmul(
                        ps_[:],
                        kt[:, k * 128 : (k + 1) * 128],
                        qt[:, k * R : (k + 1) * R],
                        start=(k == 0), stop=(k == KC - 1),
                    )
                ex = ep.tile([128, R], F32, name=f"e{j}", tag="e")
                nc.scalar.activation(ex[:], ps_[:], mybir.ActivationFunctionType.Exp)
                pt = ppool.tile([128, R], F32R, name=f"pt{j}", tag="pt")
                nc.vector.scalar_tensor_tensor(
                    pt[:], io[:], th[:, j : j + 1], ex[:],
                    op0=mybir.AluOpType.is_le, op1=mybir.AluOpType.mult,
                )
                pts.append(pt)
            # ---- read^T = V_aug^T @ P^T: V stationary, P^T moving (full 512) ----
            accs = {}
            for q in range(NQ):
                vts = []
                for jj in range(JQ):
                    j = q * JQ + jj
                    vt = vp.tile([128, VA], F32R, name=f"vt{j}", tag="vt")
                    nc.sync.dma_start(vt[:], v_in[:, j * VA : (j + 1) * VA])
                    vts.append(vt)
                for n in range(NN + 1):  # 8 feature chunks + (ones, pad) chunk
                    c0, w = (n * 128, 128) if n < NN else (D, 2)
                    pz = p2.tile([128, R], F32, name=f"pv{q}_{n}", tag="pv")
                    for jj in range(JQ):
                        j = q * JQ + jj
                        nc.tensor.matmul(
                            pz[:w, :],
                            vts[jj][:, c0 : c0 + w],
                            pts[j][:],
                            start=(jj == 0), stop=(jj == JQ - 1),
                        )
                    if q == 0:
                        a_ = ac.tile([128, R], F32, name=f"acc{n}", tag="ac")
                        accs[n] = a_
                        nc.vector.tensor_copy(a_[:w, :], pz[:w, :])
                    else:
                        a_ = accs[n]
                        nc.vector.tensor_add(a_[:w, :], a_[:w, :], pz[:w, :])
            # ---- ship unnormalized read^T + denominator row; host divides ----
            for n in range(NN):
                nc.sync.dma_start(rdT[n * 128 : (n + 1) * 128, :], accs[n][:])
            nc.sync.dma_start(rdT[D : D + 2, :], accs[NN][:2, :])
    nc.finalize()
    return nc


def _chunk_rows(a, nchunks):
    # [nchunks*128, C] -> [128, nchunks*C] with [p, k*C+c] = a[128k+p, c]
    n, c = a.shape
    assert n == nchunks * 128
    return np.ascontiguousarray(
        a.reshape(nchunks, 128, c).transpose(1, 0, 2).reshape(128, nchunks * c)
    )


def kernel(x, Wk, bk, Wq, bq, Wv, bv):
    x = np.asarray(x, dtype=np.float32)
    Wk = np.asarray(Wk, dtype=np.float32)
    Wq = np.asarray(Wq, dtype=np.float32)
    Wv = np.asarray(Wv, dtype=np.float32)
    bk = np.asarray(bk, dtype=np.float32)
    bq = np.asarray(bq, dtype=np.float32)
    bv = np.asarray(bv, dtype=np.float32)

    sc = np.float32(1.0 / np.sqrt(D))
    if "p1" not in _cache:
        _cache["p1"] = _build_phase1()
    if "p2" not in _cache:
        _cache["p2"] = _build_phase2()

    wq_in = _chunk_rows(Wq * sc, KC)
    wk_in = _chunk_rows(Wk, KC)
    wv_in = _chunk_rows(Wv, KC)
    in_maps1 = []
    for c in range(NCORES):
        xs = x[c * R : (c + 1) * R]
        xsT_in = _chunk_rows(np.ascontiguousarray(xs.T), KC)
        in_maps1.append({"xsT": xsT_in, "wq": wq_in, "wk": wk_in, "wv": wv_in})
    res1 = run_bass_kernel_spmd(_cache["p1"], in_maps1, list(range(NCORES))).results

    bq_s = (bq * sc)[None, :]
    qs = [res1[c]["q"] + bq_s for c in range(NCORES)]
    k_g = np.concatenate([res1[c]["k"] for c in range(NCORES)], axis=0) + bk[None, :]
    v_g = np.concatenate([res1[c]["v"] for c in range(NCORES)], axis=0) + bv[None, :]
    kT_g = np.ascontiguousarray(k_g.T)  # [D, S]
    v_aug = np.concatenate(
        [v_g, np.ones((S, 1), np.float32), np.zeros((S, 1), np.float32)], axis=1
    )

    # kt layout: [p, j, k, c] = kT_g[128k+p, 128j+c]
    kt_in = np.ascontiguousarray(
        kT_g.reshape(KC, 128, J, 128).transpose(1, 2, 0, 3).reshape(128, J * D)
    )
    v_in = _chunk_rows(v_aug, J)
    io_in = np.ascontiguousarray(
        np.broadcast_to(np.arange(R, dtype=np.float32), (128, R))
    )
    p_idx = np.arange(128, dtype=np.float32)[:, None]
    j_idx = np.arange(J, dtype=np.float32)[None, :]
    in_maps2 = []
    for c in range(NCORES):
        thr_c = np.ascontiguousarray(128.0 * j_idx + p_idx - 512.0 * c).astype(np.float32)
        in_maps2.append({
            "qt": _chunk_rows(np.ascontiguousarray(qs[c].T), KC),
            "kt": kt_in,
            "vi": v_in,
            "io": io_in,
            "th": thr_c,
        })
    res2 = run_bass_kernel_spmd(_cache["p2"], in_maps2, list(range(NCORES))).results

    read = np.concatenate(
        [(res2[c]["rdT"][:D] / res2[c]["rdT"][D : D + 1]).T for c in range(NCORES)], axis=0
    )
    return np.concatenate([x, read], axis=1)
